# revision 50
# baseline (speedup 1.0000x reference)
"""Causal MHA + RoPE on 8 TRN2 NeuronCores — v7.

Sharding: 8 cores = 2 batch x 4 head-groups; each core does 4 heads of one
batch (QKV proj sliced to EL=256 out dims, full attention for those heads,
partial O-proj summed on host, y in bf16).

Scheduling model: engines execute their instruction queues IN ORDER, so
emission order is execution order per engine. The attention phase is paced
by the scalar-engine exp stream (~1us per key-tile); the PE has ~0.5us of
idle per tile under it. All j2=1 projection work and the output projection
are therefore chopped into <=3-matmul "filler pieces" injected one per
attention tile, with deadlines enforced by force-drain before the phase
that needs their results.

PSUM budget (8 banks): "big" 2x[128,1024] (scores double-buffer; also the
j2=0 projection chains before attention starts), "accA" 2x[128,512]
(AV accumulators with ones-row denominator), "accB" 2x[128,512]
(filler chains / output-projection tiles / warmup).
"""
import math
import numpy as np
import ml_dtypes

import concourse.bass as bass
import concourse.mybir as mybir
import concourse.tile as tile
from concourse import bacc
from concourse.bass import ds
from concourse.bass_utils import run_bass_kernel_spmd

F32 = mybir.dt.float32
BF16 = mybir.dt.bfloat16
EXP = mybir.ActivationFunctionType.Exp

D_MODEL = 1024
DK = 64
THETA = 10000.0
B, S = 2, 2048
HPC = 4
EL = HPC * DK
SCALE = 1.0 / math.sqrt(DK)
NQ = 512
NT = 128
NKT = S // NT
DCH = D_MODEL // 128
VW = DK + 1

_CACHE = {}


def _build_nc():
    nc = bacc.Bacc(None, target_bir_lowering=False)
    xT = nc.declare_dram_parameter("xT", [D_MODEL, S], BF16, isOutput=False)
    wq = nc.declare_dram_parameter("wq", [D_MODEL, EL], BF16, isOutput=False)
    wk = nc.declare_dram_parameter("wk", [D_MODEL, EL], BF16, isOutput=False)
    wv = nc.declare_dram_parameter("wv", [D_MODEL, EL], BF16, isOutput=False)
    wo = nc.declare_dram_parameter("wo", [EL, D_MODEL], BF16, isOutput=False)
    cosT = nc.declare_dram_parameter("cosT", [128, S], BF16, isOutput=False)
    sinT = nc.declare_dram_parameter("sinT", [128, S], BF16, isOutput=False)
    y = nc.declare_dram_parameter("y", [S, D_MODEL], BF16, isOutput=True)

    with tile.TileContext(nc) as tc:
        with (
            tc.tile_pool(name="p_fin", bufs=1) as p_fin,
            tc.tile_pool(name="p_work", bufs=1) as p_work,
            tc.tile_pool(name="ps", bufs=1, space="PSUM") as ps,
        ):
            # ---- persistent tiles ----
            qt_fin = p_fin.tile([128, 2 * S], BF16, tag="qt_fin", name="qt_fin")
            kt_fin = p_fin.tile([128, 2 * S], BF16, tag="kt_fin", name="kt_fin")
            v_aug = p_fin.tile([128, NKT * HPC * VW], BF16, tag="v_aug", name="v_aug")
            attnT = [p_fin.tile([128, S], BF16, tag=f"attnT{p}", name=f"attnT{p}")
                     for p in range(2)]

            # ---- PE warmup: junk matmuls with no DMA deps ----
            junk = p_fin.tile([128, NQ], BF16, tag="junk", name="junk")
            nc.vector.memset(junk, 0.125)
            for _ in range(12):
                wps = ps.tile([128, NQ], F32, tag="accB", bufs=2, name="wps")
                nc.tensor.matmul(wps[:, 0:NQ], junk[:, 0:128], junk[:, 0:NQ],
                                 start=True, stop=True)

            # ---- input DMAs ----
            xp = {}
            for j2 in range(2):
                xp[j2] = [p_fin.tile([128, 1024], BF16, tag="xt", bufs=16,
                                     name=f"x{j2}_{d}") for d in range(DCH)]
            wq_sb = p_fin.tile([128, DCH * EL], BF16, tag="wq", name="wq_sb")
            wk_sb = p_fin.tile([128, DCH * EL], BF16, tag="wk", name="wk_sb")
            wv_sb = p_fin.tile([128, DCH * EL], BF16, tag="wv", name="wv_sb")
            cos_sb = p_fin.tile([128, S], BF16, tag="cos", name="cos_sb")
            sin_sb = p_fin.tile([128, S], BF16, tag="sin", name="sin_sb")
            wo_sb = p_fin.tile([128, 2 * D_MODEL], BF16, tag="wo", name="wo_sb")

            def dma_x(j2, d, eng):
                eng.dma_start(out=xp[j2][d],
                              in_=xT[d * 128:(d + 1) * 128,
                                     j2 * 1024:(j2 + 1) * 1024])

            def dma_w_half(w_sb, w, c, eng):
                eng.dma_start(
                    out=w_sb.rearrange("p (d c f) -> p d c f", d=DCH, c=2)[:, :, c],
                    in_=w.rearrange("(d p) (c f) -> p d c f", p=128, c=2)[:, :, c])

            # j2=0 critical loads split across the two HWDGE queues; the
            # j2=1 x and cos/sin go on gpsimd AFTER the b0 swap triggers,
            # whose sem-waits delay them past the startup bandwidth crunch.
            dma_x(0, 1, nc.scalar)
            dma_x(0, 3, nc.scalar)
            dma_w_half(wq_sb, wq, 0, nc.scalar)
            dma_x(0, 5, nc.scalar)
            dma_x(0, 7, nc.scalar)
            nc.scalar.dma_start(out=cos_sb[:, 0:1024], in_=cosT[:, 0:1024])
            nc.scalar.dma_start(out=sin_sb[:, 0:1024], in_=sinT[:, 0:1024])
            dma_w_half(wq_sb, wq, 1, nc.scalar)
            nc.scalar.dma_start(out=cos_sb[:, 1024:2048], in_=cosT[:, 1024:2048])
            nc.scalar.dma_start(out=sin_sb[:, 1024:2048], in_=sinT[:, 1024:2048])
            for d in (0, 2, 4, 6):
                dma_x(0, d, nc.sync)
            for c in range(2):
                dma_w_half(wk_sb, wk, c, nc.sync)
            nc.sync.dma_start(out=wv_sb.rearrange("p (d e) -> p d e", d=DCH),
                              in_=wv.rearrange("(d p) e -> p d e", p=128))
            # j2=1 x sits behind the j2=0 loads in the same FIFO, so it
            # cannot steal HBM bandwidth from the startup critical path
            # (Tile hoists dep-free DMAs, so a separate queue won't wait).
            for d in range(DCH):
                dma_x(1, d, nc.sync)
            nc.sync.dma_start(out=wo_sb.rearrange("p (c e) -> p c e", c=2),
                              in_=wo.rearrange("(c p) e -> p c e", p=128))

            def dma_late_inputs():
                pass

            # bridge junk matmuls: keep the PE warm across the HBM-bound
            # startup window until the first x chunks land (their own cost
            # is hidden inside the DMA wait)
            for _ in range(20):
                wps = ps.tile([128, NQ], F32, tag="accB", bufs=2, name="wb")
                nc.tensor.matmul(wps[:, 0:256], junk[:, 0:128], junk[:, 0:256],
                                 start=True, stop=True)

            # v_aug: set everything to 1.0 once; V copies overwrite [0:DK].
            nc.vector.memset(v_aug, 1.0)

            # shared causal triangle for the diagonal wedge band
            tri = p_fin.tile([128, 2 * NT], BF16, tag="tri", name="tri")
            nc.vector.memset(tri, 1.0)
            nc.gpsimd.affine_select(
                out=tri.rearrange("p (h b) -> p h b", h=2),
                in_=tri.rearrange("p (h b) -> p h b", h=2),
                compare_op=mybir.AluOpType.is_ge,
                fill=0.0, base=0,
                pattern=[[0, 2], [1, NT]],
                channel_multiplier=-1,
            )

            # ---- j2=0 projections: c=0 kinds first, swap-adds deferred ----
            u2b = {}

            def get_u2b(kind, c):
                key = (kind, c)
                if key not in u2b:
                    u2b[key] = (
                        p_work.tile([128, 1024], BF16, tag="u", bufs=4,
                                    name=f"ub{kind}{c}"),
                        p_work.tile([128, 1024], BF16, tag="us", bufs=4,
                                    name=f"usb{kind}{c}"))
                return u2b[key]

            def emit_b0():
                for kind, c in ((0, 0), (1, 0), (0, 1), (1, 1)):
                    w_sb = wq_sb if kind == 0 else wk_sb
                    fin = qt_fin if kind == 0 else kt_fin
                    u2, _ = get_u2b(kind, c)
                    pq = ps.tile([128, 1024], F32, tag="big", bufs=2, name="pq")
                    for h5 in range(2):
                        for d in range(DCH):
                            nc.tensor.matmul(
                                pq[:, ds(h5 * NQ, NQ)],
                                w_sb[:, ds(d * EL + c * 128, 128)],
                                xp[0][d][:, ds(h5 * NQ, NQ)],
                                start=(d == 0), stop=(d == DCH - 1))
                    raw = p_work.tile([128, 1024], BF16, tag="raw", bufs=4,
                                      name="raw")
                    nc.vector.tensor_copy(raw, pq)
                    nc.vector.tensor_mul(u2, raw, sin_sb[:, 0:1024])
                    nc.vector.tensor_mul(fin[:, ds(c * S, 1024)],
                                         raw, cos_sb[:, 0:1024])
                    us2 = get_u2b(kind, c)[1]
                    for blk in range(4):
                        srcb = (blk ^ 1) * 32
                        nc.gpsimd.dma_start(out=us2[blk * 32:(blk + 1) * 32, :],
                                            in_=u2[srcb:srcb + 32, :])
                for kind, c in ((0, 0), (1, 0), (0, 1), (1, 1)):
                    fin = qt_fin if kind == 0 else kt_fin
                    u2, us2 = get_u2b(kind, c)
                    nc.vector.tensor_add(fin[:, ds(c * S, 1024)],
                                         fin[:, ds(c * S, 1024)], us2)
                dma_late_inputs()

            # ---- filler pieces: <=3 PE matmuls each ----
            u2k = {}

            def qk_half_pieces(kind, c, h5):
                """j2=1 projection half-chain as 3 pieces (3+3+2 MMs)."""
                w_sb = wq_sb if kind == 0 else wk_sb
                fin = qt_fin if kind == 0 else kt_fin
                st = {}

                def mm(d):
                    nc.tensor.matmul(
                        st["pq"][:, 0:NQ],
                        w_sb[:, ds(d * EL + c * 128, 128)],
                        xp[1][d][:, ds(h5 * NQ, NQ)],
                        start=(d == 0), stop=(d == DCH - 1))

                def p1():
                    st["pq"] = ps.tile([128, NQ], F32, tag="accB", bufs=2,
                                       name="pqh")
                    for d in range(3):
                        mm(d)

                def p2():
                    for d in range(3, 6):
                        mm(d)

                def p3():
                    for d in range(6, 8):
                        mm(d)
                    if kind not in u2k:
                        u2k[kind] = (
                            p_work.tile([128, 2048], BF16, tag="u", bufs=4,
                                        name=f"u2j1_{kind}"),
                            p_work.tile([128, 2048], BF16, tag="us", bufs=4,
                                        name=f"us2j1_{kind}"))
                    u2, _ = u2k[kind]
                    rawh = p_work.tile([128, NQ], BF16, tag="rawh", bufs=4,
                                       name="rawh")
                    nc.vector.tensor_copy(rawh, st["pq"])
                    ssl = ds(1024 + h5 * NQ, NQ)
                    nc.vector.tensor_mul(u2[:, ds(c * 1024 + h5 * NQ, NQ)],
                                         rawh, sin_sb[:, ssl])
                    nc.vector.tensor_mul(
                        fin[:, ds(c * S + 1024 + h5 * NQ, NQ)],
                        rawh, cos_sb[:, ssl])

                return [p1, p2, p3]

            def swap_add_piece(kind):
                def f():
                    fin = qt_fin if kind == 0 else kt_fin
                    u2, us2 = u2k[kind]
                    for blk in range(4):
                        src = (blk ^ 1) * 32
                        nc.gpsimd.dma_start(out=us2[blk * 32:(blk + 1) * 32, :],
                                            in_=u2[src:src + 32, :])
                    for c in range(2):
                        fsl = ds(c * S + 1024, 1024)
                        nc.vector.tensor_add(fin[:, fsl], fin[:, fsl],
                                             us2[:, ds(c * 1024, 1024)])
                return [f]

            def v_pieces(j2, sti):
                """V projection chain as 3 pieces."""
                t = 8 * j2 + sti
                st = {}

                def mm(d):
                    nc.tensor.matmul(
                        st["pv"][:, 0:EL],
                        xp[j2][d][:, ds(sti * 128, 128)],
                        wv_sb[:, ds(d * EL, EL)],
                        start=(d == 0), stop=(d == DCH - 1))

                def p1():
                    st["pv"] = ps.tile([128, NQ], F32, tag="accB", bufs=2,
                                       name="pvh")
                    for d in range(3):
                        mm(d)

                def p2():
                    for d in range(3, 6):
                        mm(d)

                def p3():
                    for d in range(6, 8):
                        mm(d)
                    vview = v_aug[:, ds(t * HPC * VW, HPC * VW)].rearrange(
                        "p (h a) -> p h a", a=VW)
                    nc.vector.tensor_copy(
                        vview[:, :, 0:DK],
                        st["pv"][:, 0:EL].rearrange("p (h m) -> p h m", m=DK))

                return [p1, p2, p3]

            ysb_st = {}

            def e_pieces(sti):
                """output projection row-block as 2 pieces (2 MMs each)."""
                def ep(e2):
                    def f():
                        if e2 == 0:
                            ysb_st[sti] = p_work.tile([128, 1024], BF16,
                                                      tag="ysb", bufs=3,
                                                      name="ysb")
                        ysb = ysb_st[sti]
                        py = ps.tile([128, NQ], F32, tag="accB", bufs=2,
                                     name="py")
                        for c in range(2):
                            nc.tensor.matmul(
                                py[:, 0:NQ],
                                attnT[c][:, ds(sti * 128, 128)],
                                wo_sb[:, ds(c * D_MODEL + e2 * NQ, NQ)],
                                start=(c == 0), stop=(c == 1))
                        nc.vector.tensor_copy(ysb[:, ds(e2 * NQ, NQ)],
                                              py[:, 0:NQ])
                        if e2 == 1:
                            eng = nc.sync if sti % 2 == 0 else nc.gpsimd
                            eng.dma_start(out=y[sti * 128:(sti + 1) * 128, :],
                                          in_=ysb)
                    return f
                return [ep(0), ep(1)]

            # (deadline_phase, min_phase, piece) list; phases 0..7 =
            # A(0,0),A(1,0),A(0,1),A(1,1),A(0,2),A(1,2),A(0,3),A(1,3).
            # A piece may only run in phases [min_phase, deadline]: the
            # deadline is force-drained; min_phase prevents consuming an
            # e-piece before the attnT span it reads has been written
            # (Tile would order it before the write -> garbage).
            FILL = []
            for sti in range(4, 8):
                FILL += [(2, 0, p) for p in v_pieces(0, sti)]
            for c in range(2):
                for h5 in range(2):
                    FILL += [(4, 0, p) for p in qk_half_pieces(0, c, h5)]
            FILL += [(4, 0, p) for p in swap_add_piece(0)]
            for c in range(2):
                for h5 in range(2):
                    FILL += [(4, 0, p) for p in qk_half_pieces(1, c, h5)]
            FILL += [(4, 0, p) for p in swap_add_piece(1)]
            for sti in range(4):
                FILL += [(4, 0, p) for p in v_pieces(1, sti)]
            for sti in range(4, 8):
                FILL += [(6, 0, p) for p in v_pieces(1, sti)]
            for sti in range(4):
                FILL += [(99, 2, p) for p in e_pieces(sti)]
            for sti in range(4, 8):
                FILL += [(99, 4, p) for p in e_pieces(sti)]
            for sti in range(8, 12):
                FILL += [(99, 6, p) for p in e_pieces(sti)]
            fptr = [0]
            cur_phase = [0]

            def keepalive():
                # tiny junk matmul: keeps the PE HAM activity window busy so
                # the clock never re-throttles when real work briefly dries up
                wps = ps.tile([128, NQ], F32, tag="accB", bufs=2, name="ka")
                nc.tensor.matmul(wps[:, 0:128], junk[:, 0:128], junk[:, 0:128],
                                 start=True, stop=True)

            def pop_filler():
                if fptr[0] < len(FILL) and FILL[fptr[0]][1] <= cur_phase[0]:
                    FILL[fptr[0]][2]()
                    fptr[0] += 1
                else:
                    keepalive()

            def drain(phase):
                while fptr[0] < len(FILL) and FILL[fptr[0]][0] <= phase:
                    FILL[fptr[0]][2]()
                    fptr[0] += 1

            # ---- attention phase ----
            def emit_attn(p, j, phase, local_fill=None):
                cur_phase[0] = phase
                drain(phase)
                ntile = 4 * j + 4
                pva = ps.tile([128, NQ], F32, tag="accA", bufs=2, name="pva")
                pvb = ps.tile([128, NQ], F32, tag="accA", bufs=2, name="pvb")

                def emit_scores_exp(t):
                    dd = max(0, t - 4 * j)
                    q0 = dd * NT
                    w = NQ - q0
                    stp = ps.tile([128, 2 * NQ], F32, tag="big", bufs=2,
                                  name="stp")
                    for hh in range(2):
                        nc.tensor.matmul(
                            stp[:, ds(hh * NQ + q0, w)],
                            kt_fin[hh * 64:(hh + 1) * 64, ds(p * S + t * NT, NT)],
                            qt_fin[hh * 64:(hh + 1) * 64,
                                   ds(p * S + j * NQ + q0, w)],
                            start=True, stop=True)
                    ste = p_work.tile([128, 2 * NQ], BF16, tag="ste", bufs=8,
                                      name="ste")
                    if t >= 4 * j:
                        stp_v = stp.rearrange("p (h q) -> p h q", h=2)[:, :, q0:NQ]
                        ste_v = ste.rearrange("p (h q) -> p h q", h=2)[:, :, q0:NQ]
                        nc.scalar.activation(ste_v, stp_v, EXP, scale=SCALE)
                        band = ste.rearrange("p (h q) -> p h q", h=2)[:, :,
                                                                      q0:q0 + NT]
                        nc.vector.tensor_mul(
                            band, band, tri.rearrange("p (h b) -> p h b", h=2))
                    else:
                        nc.scalar.activation(ste, stp, EXP, scale=SCALE)
                    return ste

                def emit_av(t, ste):
                    dd = max(0, t - 4 * j)
                    q0 = dd * NT
                    w = NQ - q0
                    for hh, pvx in ((0, pva), (1, pvb)):
                        nc.tensor.matmul(
                            pvx[0:VW, ds(q0, w)],
                            v_aug[:, ds(t * HPC * VW + (2 * p + hh) * VW, VW)],
                            ste[:, ds(hh * NQ + q0, w)],
                            start=(t == 0), stop=(t == ntile - 1))

                pending = []
                for t in range(ntile):
                    ste = emit_scores_exp(t)
                    # phase-0 special: V chains injected between scores and
                    # AV so the first exps overlap the V projections
                    if local_fill:
                        for _ in range(3):
                            if local_fill:
                                local_fill.pop(0)()
                    pending.append((t, ste))
                    if len(pending) > 1:
                        (pt, pste) = pending.pop(0)
                        emit_av(pt, pste)
                        pop_filler()
                        if phase < 4 and fptr[0] < len(FILL):
                            pop_filler()
                for (pt, pste) in pending:
                    emit_av(pt, pste)
                    pop_filler()

                lcp_a = p_work.tile([1, NQ], F32, tag="lcp_a", bufs=3, name="lcp_a")
                lcp_b = p_work.tile([1, NQ], F32, tag="lcp_b", bufs=3, name="lcp_b")
                nc.vector.tensor_copy(lcp_a, pva[64:65, :])
                nc.vector.tensor_copy(lcp_b, pvb[64:65, :])
                recl_a = p_work.tile([1, NQ], F32, tag="recl_a", bufs=3, name="recl_a")
                recl_b = p_work.tile([1, NQ], F32, tag="recl_b", bufs=3, name="recl_b")
                nc.vector.reciprocal_approx_fast(recl_a, lcp_a)
                nc.vector.reciprocal_approx_fast(recl_b, lcp_b)
                rb_a = p_work.tile([64, NQ], F32, tag="rb_a", bufs=3, name="rb_a")
                rb_b = p_work.tile([64, NQ], F32, tag="rb_b", bufs=3, name="rb_b")
                nc.gpsimd.partition_broadcast(rb_a, recl_a, channels=64)
                nc.gpsimd.partition_broadcast(rb_b, recl_b, channels=64)
                sl = ds(j * NQ, NQ)
                nc.vector.tensor_mul(attnT[p][0:64, sl], pva[0:64, :], rb_a)
                nc.vector.tensor_mul(attnT[p][64:128, sl], pvb[0:64, :], rb_b)

            emit_b0()
            v03 = []
            for sti in range(4):
                v03 += v_pieces(0, sti)
            phase = 0
            for j in range(4):
                for p in range(2):
                    emit_attn(p, j, phase,
                              local_fill=v03 if (p == 0 and j == 0) else None)
                    phase += 1
            # tail: e(3) on the (now free) big pool, psum evict on the (now
            # idle) scalar engine, plus any unconsumed fillers
            # keepalives: the final norm chain stalls the PE ~5us, which
            # would re-throttle the clock and run e(3) at half rate
            for _ in range(10):
                keepalive()
            drain(99)
            for sti in range(12, 16):
                py2 = ps.tile([128, 1024], F32, tag="big", bufs=2, name="py2")
                for e2 in range(2):
                    for c in range(2):
                        nc.tensor.matmul(
                            py2[:, ds(e2 * NQ, NQ)],
                            attnT[c][:, ds(sti * 128, 128)],
                            wo_sb[:, ds(c * D_MODEL + e2 * NQ, NQ)],
                            start=(c == 0), stop=(c == 1))
                ysb = p_work.tile([128, 1024], BF16, tag="ysb", bufs=3,
                                  name="ysb")
                if sti % 2 == 0:
                    nc.scalar.copy(ysb, py2)
                else:
                    nc.vector.tensor_copy(ysb, py2)
                eng = nc.sync if sti % 2 == 0 else nc.gpsimd
                eng.dma_start(out=y[sti * 128:(sti + 1) * 128, :], in_=ysb)
    nc.finalize()
    return nc


def _host_prep(x, Wq, Wk, Wv, Wo):
    x = np.asarray(x, dtype=np.float32)
    Wq, Wk, Wv, Wo = (np.asarray(w, dtype=np.float32) for w in (Wq, Wk, Wv, Wo))
    bf = ml_dtypes.bfloat16

    p64 = np.concatenate([np.arange(0, DK, 2), np.arange(1, DK, 2)])
    freqs = 1.0 / THETA ** (np.arange(0, DK, 2, dtype=np.float64) / DK)
    ang = np.arange(S, dtype=np.float64)[None, :] * freqs[:, None]
    cos32 = np.cos(ang).astype(np.float32)
    sin32 = np.sin(ang).astype(np.float32)
    cosT = np.ascontiguousarray(np.tile(cos32, (4, 1))).astype(bf)
    # sin_alt: pre-swap layout [s, -s, s, -s]; multiply first, then swap
    # 32-blocks, landing [-s, s, -s, s] contributions.
    sinT = np.ascontiguousarray(
        np.concatenate([sin32, -sin32, sin32, -sin32], axis=0)).astype(bf)

    xTs = [np.ascontiguousarray(x[b].T).astype(bf) for b in range(B)]
    perm = np.concatenate([h * DK + p64 for h in range(HPC)])

    in_maps = []
    for core in range(8):
        bg, hg = core // 4, core % 4
        sl = slice(hg * EL, (hg + 1) * EL)
        in_maps.append({
            "xT": xTs[bg],
            "wq": np.ascontiguousarray(Wq[sl][perm].T).astype(bf),
            "wk": np.ascontiguousarray(Wk[sl][perm].T).astype(bf),
            "wv": np.ascontiguousarray(Wv[sl].T).astype(bf),
            "wo": np.ascontiguousarray(Wo[:, sl].T).astype(bf),
            "cosT": cosT,
            "sinT": sinT,
        })
    return in_maps


def kernel(x, Wq, Wk, Wv, Wo, _trace=False):
    if "nc" not in _CACHE:
        _CACHE["nc"] = _build_nc()
    nc = _CACHE["nc"]
    in_maps = _host_prep(x, Wq, Wk, Wv, Wo)
    res = run_bass_kernel_spmd(nc, in_maps, core_ids=list(range(8)), trace=_trace)
    _CACHE["last_result"] = res
    out = np.zeros((B, S, D_MODEL), dtype=np.float32)
    for core in range(8):
        out[core // 4] += np.asarray(res.results[core]["y"], dtype=np.float32)
    return out


# revision 52
# speedup vs baseline: 1.0323x; 1.0323x over previous
"""Causal MHA + RoPE on 8 TRN2 NeuronCores — v7.

Sharding: 8 cores = 2 batch x 4 head-groups; each core does 4 heads of one
batch (QKV proj sliced to EL=256 out dims, full attention for those heads,
partial O-proj summed on host, y in bf16).

Scheduling model: engines execute their instruction queues IN ORDER, so
emission order is execution order per engine. The attention phase is paced
by the scalar-engine exp stream (~1us per key-tile); the PE has ~0.5us of
idle per tile under it. All j2=1 projection work and the output projection
are therefore chopped into <=3-matmul "filler pieces" injected one per
attention tile, with deadlines enforced by force-drain before the phase
that needs their results.

PSUM budget (8 banks): "big" 2x[128,1024] (scores double-buffer; also the
j2=0 projection chains before attention starts), "accA" 2x[128,512]
(AV accumulators with ones-row denominator), "accB" 2x[128,512]
(filler chains / output-projection tiles / warmup).
"""
import math
import numpy as np
import ml_dtypes

import concourse.bass as bass
import concourse.mybir as mybir
import concourse.tile as tile
from concourse import bacc
from concourse.bass import ds
from concourse.bass_utils import run_bass_kernel_spmd

F32 = mybir.dt.float32
BF16 = mybir.dt.bfloat16
EXP = mybir.ActivationFunctionType.Exp

D_MODEL = 1024
DK = 64
THETA = 10000.0
B, S = 2, 2048
HPC = 4
EL = HPC * DK
SCALE = 1.0 / math.sqrt(DK)
NQ = 512
NT = 128
NKT = S // NT
DCH = D_MODEL // 128
VW = DK + 1

_CACHE = {}


def _build_nc():
    nc = bacc.Bacc(None, target_bir_lowering=False)
    xT = nc.declare_dram_parameter("xT", [D_MODEL, S], BF16, isOutput=False)
    wq = nc.declare_dram_parameter("wq", [D_MODEL, EL], BF16, isOutput=False)
    wk = nc.declare_dram_parameter("wk", [D_MODEL, EL], BF16, isOutput=False)
    wv = nc.declare_dram_parameter("wv", [D_MODEL, EL], BF16, isOutput=False)
    wo = nc.declare_dram_parameter("wo", [EL, D_MODEL], BF16, isOutput=False)
    cosT = nc.declare_dram_parameter("cosT", [128, S], BF16, isOutput=False)
    sinT = nc.declare_dram_parameter("sinT", [128, S], BF16, isOutput=False)
    y = nc.declare_dram_parameter("y", [S, D_MODEL], BF16, isOutput=True)

    with tile.TileContext(nc) as tc:
        with (
            tc.tile_pool(name="p_fin", bufs=1) as p_fin,
            tc.tile_pool(name="p_work", bufs=1) as p_work,
            tc.tile_pool(name="ps", bufs=1, space="PSUM") as ps,
        ):
            # ---- persistent tiles ----
            qt_fin = p_fin.tile([128, 2 * S], BF16, tag="qt_fin", name="qt_fin")
            kt_fin = p_fin.tile([128, 2 * S], BF16, tag="kt_fin", name="kt_fin")
            v_aug = p_fin.tile([128, NKT * HPC * VW], BF16, tag="v_aug", name="v_aug")
            attnT = [p_fin.tile([128, S], BF16, tag=f"attnT{p}", name=f"attnT{p}")
                     for p in range(2)]

            # ---- PE warmup: junk matmuls with no DMA deps ----
            junk = p_fin.tile([128, NQ], BF16, tag="junk", name="junk")
            nc.vector.memset(junk, 0.125)
            for _ in range(12):
                wps = ps.tile([128, NQ], F32, tag="accB", bufs=2, name="wps")
                nc.tensor.matmul(wps[:, 0:NQ], junk[:, 0:128], junk[:, 0:NQ],
                                 start=True, stop=True)

            # ---- input DMAs ----
            xp = {}
            for j2 in range(2):
                xp[j2] = [p_fin.tile([128, 1024], BF16, tag="xt", bufs=16,
                                     name=f"x{j2}_{d}") for d in range(DCH)]
            wq_sb = p_fin.tile([128, DCH * EL], BF16, tag="wq", name="wq_sb")
            wk_sb = p_fin.tile([128, DCH * EL], BF16, tag="wk", name="wk_sb")
            wv_sb = p_fin.tile([128, DCH * EL], BF16, tag="wv", name="wv_sb")
            cos_sb = p_fin.tile([128, S], BF16, tag="cos", name="cos_sb")
            sin_sb = p_fin.tile([128, S], BF16, tag="sin", name="sin_sb")
            wo_sb = p_fin.tile([128, 2 * D_MODEL], BF16, tag="wo", name="wo_sb")

            def dma_x(j2, d, eng):
                eng.dma_start(out=xp[j2][d],
                              in_=xT[d * 128:(d + 1) * 128,
                                     j2 * 1024:(j2 + 1) * 1024])

            def dma_w_half(w_sb, w, c, eng):
                eng.dma_start(
                    out=w_sb.rearrange("p (d c f) -> p d c f", d=DCH, c=2)[:, :, c],
                    in_=w.rearrange("(d p) (c f) -> p d c f", p=128, c=2)[:, :, c])

            # j2=0 critical loads split across the two HWDGE queues; the
            # j2=1 x and cos/sin go on gpsimd AFTER the b0 swap triggers,
            # whose sem-waits delay them past the startup bandwidth crunch.
            dma_x(0, 1, nc.scalar)
            dma_x(0, 3, nc.scalar)
            dma_w_half(wq_sb, wq, 0, nc.scalar)
            dma_x(0, 5, nc.scalar)
            dma_x(0, 7, nc.scalar)
            nc.scalar.dma_start(out=cos_sb[:, 0:1024], in_=cosT[:, 0:1024])
            nc.scalar.dma_start(out=sin_sb[:, 0:1024], in_=sinT[:, 0:1024])
            dma_w_half(wq_sb, wq, 1, nc.scalar)
            nc.scalar.dma_start(out=cos_sb[:, 1024:2048], in_=cosT[:, 1024:2048])
            nc.scalar.dma_start(out=sin_sb[:, 1024:2048], in_=sinT[:, 1024:2048])
            for d in (0, 2, 4, 6):
                dma_x(0, d, nc.sync)
            for c in range(2):
                dma_w_half(wk_sb, wk, c, nc.sync)
            nc.sync.dma_start(out=wv_sb.rearrange("p (d e) -> p d e", d=DCH),
                              in_=wv.rearrange("(d p) e -> p d e", p=128))
            # j2=1 x sits behind the j2=0 loads in the same FIFO, so it
            # cannot steal HBM bandwidth from the startup critical path
            # (Tile hoists dep-free DMAs, so a separate queue won't wait).
            for d in range(DCH):
                dma_x(1, d, nc.sync)
            nc.sync.dma_start(out=wo_sb.rearrange("p (c e) -> p c e", c=2),
                              in_=wo.rearrange("(c p) e -> p c e", p=128))

            def dma_late_inputs():
                pass

            # bridge junk matmuls: keep the PE warm across the HBM-bound
            # startup window until the first x chunks land (their own cost
            # is hidden inside the DMA wait)
            for _ in range(20):
                wps = ps.tile([128, NQ], F32, tag="accB", bufs=2, name="wb")
                nc.tensor.matmul(wps[:, 0:256], junk[:, 0:128], junk[:, 0:256],
                                 start=True, stop=True)

            # v_aug: set everything to 1.0 once; V copies overwrite [0:DK].
            nc.vector.memset(v_aug, 1.0)

            # shared causal triangle for the diagonal wedge band
            tri = p_fin.tile([128, 2 * NT], BF16, tag="tri", name="tri")
            nc.vector.memset(tri, 1.0)
            nc.gpsimd.affine_select(
                out=tri.rearrange("p (h b) -> p h b", h=2),
                in_=tri.rearrange("p (h b) -> p h b", h=2),
                compare_op=mybir.AluOpType.is_ge,
                fill=0.0, base=0,
                pattern=[[0, 2], [1, NT]],
                channel_multiplier=-1,
            )

            # ---- j2=0 projections: c=0 kinds first, swap-adds deferred ----
            u2b = {}

            def get_u2b(kind, c):
                key = (kind, c)
                if key not in u2b:
                    u2b[key] = (
                        p_work.tile([128, 1024], BF16, tag="u", bufs=4,
                                    name=f"ub{kind}{c}"),
                        p_work.tile([128, 1024], BF16, tag="us", bufs=4,
                                    name=f"usb{kind}{c}"))
                return u2b[key]

            def emit_b0():
                for kind, c in ((0, 0), (1, 0), (0, 1), (1, 1)):
                    w_sb = wq_sb if kind == 0 else wk_sb
                    fin = qt_fin if kind == 0 else kt_fin
                    u2, _ = get_u2b(kind, c)
                    pq = ps.tile([128, 1024], F32, tag="big", bufs=2, name="pq")
                    for h5 in range(2):
                        for d in range(DCH):
                            nc.tensor.matmul(
                                pq[:, ds(h5 * NQ, NQ)],
                                w_sb[:, ds(d * EL + c * 128, 128)],
                                xp[0][d][:, ds(h5 * NQ, NQ)],
                                start=(d == 0), stop=(d == DCH - 1))
                    raw = p_work.tile([128, 1024], BF16, tag="raw", bufs=4,
                                      name="raw")
                    nc.vector.tensor_copy(raw, pq)
                    nc.vector.tensor_mul(u2, raw, sin_sb[:, 0:1024])
                    nc.vector.tensor_mul(fin[:, ds(c * S, 1024)],
                                         raw, cos_sb[:, 0:1024])
                    us2 = get_u2b(kind, c)[1]
                    for blk in range(4):
                        srcb = (blk ^ 1) * 32
                        nc.gpsimd.dma_start(out=us2[blk * 32:(blk + 1) * 32, :],
                                            in_=u2[srcb:srcb + 32, :])
                for kind, c in ((0, 0), (1, 0), (0, 1), (1, 1)):
                    fin = qt_fin if kind == 0 else kt_fin
                    u2, us2 = get_u2b(kind, c)
                    nc.vector.tensor_add(fin[:, ds(c * S, 1024)],
                                         fin[:, ds(c * S, 1024)], us2)
                dma_late_inputs()

            # ---- filler pieces: <=3 PE matmuls each ----
            u2k = {}

            def qk_half_pieces(kind, c, h5):
                """j2=1 projection half-chain as 3 pieces (3+3+2 MMs)."""
                w_sb = wq_sb if kind == 0 else wk_sb
                fin = qt_fin if kind == 0 else kt_fin
                st = {}

                def mm(d):
                    nc.tensor.matmul(
                        st["pq"][:, 0:NQ],
                        w_sb[:, ds(d * EL + c * 128, 128)],
                        xp[1][d][:, ds(h5 * NQ, NQ)],
                        start=(d == 0), stop=(d == DCH - 1))

                def p1():
                    st["pq"] = ps.tile([128, NQ], F32, tag="accB", bufs=2,
                                       name="pqh")
                    for d in range(3):
                        mm(d)

                def p2():
                    for d in range(3, 6):
                        mm(d)

                def p3():
                    for d in range(6, 8):
                        mm(d)
                    if kind not in u2k:
                        u2k[kind] = (
                            p_work.tile([128, 2048], BF16, tag="u", bufs=4,
                                        name=f"u2j1_{kind}"),
                            p_work.tile([128, 2048], BF16, tag="us", bufs=4,
                                        name=f"us2j1_{kind}"))
                    u2, _ = u2k[kind]
                    rawh = p_work.tile([128, NQ], BF16, tag="rawh", bufs=4,
                                       name="rawh")
                    nc.vector.tensor_copy(rawh, st["pq"])
                    ssl = ds(1024 + h5 * NQ, NQ)
                    nc.vector.tensor_mul(u2[:, ds(c * 1024 + h5 * NQ, NQ)],
                                         rawh, sin_sb[:, ssl])
                    nc.vector.tensor_mul(
                        fin[:, ds(c * S + 1024 + h5 * NQ, NQ)],
                        rawh, cos_sb[:, ssl])

                return [p1, p2, p3]

            def swap_add_piece(kind):
                def f():
                    fin = qt_fin if kind == 0 else kt_fin
                    u2, us2 = u2k[kind]
                    for blk in range(4):
                        src = (blk ^ 1) * 32
                        nc.gpsimd.dma_start(out=us2[blk * 32:(blk + 1) * 32, :],
                                            in_=u2[src:src + 32, :])
                    for c in range(2):
                        fsl = ds(c * S + 1024, 1024)
                        nc.vector.tensor_add(fin[:, fsl], fin[:, fsl],
                                             us2[:, ds(c * 1024, 1024)])
                return [f]

            def v_pieces(j2, sti):
                """V projection chain as 3 pieces."""
                t = 8 * j2 + sti
                st = {}

                def mm(d):
                    nc.tensor.matmul(
                        st["pv"][:, 0:EL],
                        xp[j2][d][:, ds(sti * 128, 128)],
                        wv_sb[:, ds(d * EL, EL)],
                        start=(d == 0), stop=(d == DCH - 1))

                def p1():
                    st["pv"] = ps.tile([128, NQ], F32, tag="accB", bufs=2,
                                       name="pvh")
                    for d in range(3):
                        mm(d)

                def p2():
                    for d in range(3, 6):
                        mm(d)

                def p3():
                    for d in range(6, 8):
                        mm(d)
                    vview = v_aug[:, ds(t * HPC * VW, HPC * VW)].rearrange(
                        "p (h a) -> p h a", a=VW)
                    nc.vector.tensor_copy(
                        vview[:, :, 0:DK],
                        st["pv"][:, 0:EL].rearrange("p (h m) -> p h m", m=DK))

                return [p1, p2, p3]

            ysb_st = {}

            def e_pieces(sti):
                """output projection row-block as 2 pieces (2 MMs each)."""
                def ep(e2):
                    def f():
                        if e2 == 0:
                            ysb_st[sti] = p_work.tile([128, 1024], BF16,
                                                      tag="ysb", bufs=3,
                                                      name="ysb")
                        ysb = ysb_st[sti]
                        py = ps.tile([128, NQ], F32, tag="accB", bufs=2,
                                     name="py")
                        for c in range(2):
                            nc.tensor.matmul(
                                py[:, 0:NQ],
                                attnT[c][:, ds(sti * 128, 128)],
                                wo_sb[:, ds(c * D_MODEL + e2 * NQ, NQ)],
                                start=(c == 0), stop=(c == 1))
                        nc.vector.tensor_copy(ysb[:, ds(e2 * NQ, NQ)],
                                              py[:, 0:NQ])
                        if e2 == 1:
                            eng = nc.sync if sti % 2 == 0 else nc.gpsimd
                            eng.dma_start(out=y[sti * 128:(sti + 1) * 128, :],
                                          in_=ysb)
                    return f
                return [ep(0), ep(1)]

            # (deadline_phase, min_phase, piece) list; phases 0..7 =
            # A(0,0),A(1,0),A(0,1),A(1,1),A(0,2),A(1,2),A(0,3),A(1,3).
            # A piece may only run in phases [min_phase, deadline]: the
            # deadline is force-drained; min_phase prevents consuming an
            # e-piece before the attnT span it reads has been written
            # (Tile would order it before the write -> garbage).
            FILL = []
            for sti in range(4, 8):
                FILL += [(2, 0, p) for p in v_pieces(0, sti)]
            for c in range(2):
                for h5 in range(2):
                    FILL += [(4, 0, p) for p in qk_half_pieces(0, c, h5)]
            FILL += [(4, 0, p) for p in swap_add_piece(0)]
            for c in range(2):
                for h5 in range(2):
                    FILL += [(4, 0, p) for p in qk_half_pieces(1, c, h5)]
            FILL += [(4, 0, p) for p in swap_add_piece(1)]
            for sti in range(4):
                FILL += [(4, 0, p) for p in v_pieces(1, sti)]
            for sti in range(4, 8):
                FILL += [(6, 0, p) for p in v_pieces(1, sti)]
            for sti in range(4):
                FILL += [(99, 2, p) for p in e_pieces(sti)]
            for sti in range(4, 8):
                FILL += [(99, 4, p) for p in e_pieces(sti)]
            for sti in range(8, 12):
                FILL += [(99, 6, p) for p in e_pieces(sti)]
            fptr = [0]
            cur_phase = [0]

            def keepalive():
                # tiny junk matmul: keeps the PE HAM activity window busy so
                # the clock never re-throttles when real work briefly dries up
                wps = ps.tile([128, NQ], F32, tag="accB", bufs=2, name="ka")
                nc.tensor.matmul(wps[:, 0:128], junk[:, 0:128], junk[:, 0:128],
                                 start=True, stop=True)

            def pop_filler():
                if fptr[0] < len(FILL) and FILL[fptr[0]][1] <= cur_phase[0]:
                    FILL[fptr[0]][2]()
                    fptr[0] += 1
                else:
                    keepalive()

            def drain(phase):
                while fptr[0] < len(FILL) and FILL[fptr[0]][0] <= phase:
                    FILL[fptr[0]][2]()
                    fptr[0] += 1

            # ---- attention phase ----
            def emit_attn(p, j, phase, local_fill=None):
                cur_phase[0] = phase
                drain(phase)
                ntile = 4 * j + 4
                pva = ps.tile([128, NQ], F32, tag="accA", bufs=2, name="pva")
                pvb = ps.tile([128, NQ], F32, tag="accA", bufs=2, name="pvb")

                def emit_scores_exp(t):
                    dd = max(0, t - 4 * j)
                    q0 = dd * NT
                    w = NQ - q0
                    stp = ps.tile([128, 2 * NQ], F32, tag="big", bufs=2,
                                  name="stp")
                    for hh in range(2):
                        nc.tensor.matmul(
                            stp[:, ds(hh * NQ + q0, w)],
                            kt_fin[hh * 64:(hh + 1) * 64, ds(p * S + t * NT, NT)],
                            qt_fin[hh * 64:(hh + 1) * 64,
                                   ds(p * S + j * NQ + q0, w)],
                            start=True, stop=True)
                    ste = p_work.tile([128, 2 * NQ], BF16, tag="ste", bufs=8,
                                      name="ste")
                    if t >= 4 * j:
                        stp_v = stp.rearrange("p (h q) -> p h q", h=2)[:, :, q0:NQ]
                        ste_v = ste.rearrange("p (h q) -> p h q", h=2)[:, :, q0:NQ]
                        nc.scalar.activation(ste_v, stp_v, EXP, scale=SCALE)
                        band = ste.rearrange("p (h q) -> p h q", h=2)[:, :,
                                                                      q0:q0 + NT]
                        nc.vector.tensor_mul(
                            band, band, tri.rearrange("p (h b) -> p h b", h=2))
                    else:
                        nc.scalar.activation(ste, stp, EXP, scale=SCALE)
                    return ste

                def emit_av(t, ste):
                    dd = max(0, t - 4 * j)
                    q0 = dd * NT
                    w = NQ - q0
                    for hh, pvx in ((0, pva), (1, pvb)):
                        nc.tensor.matmul(
                            pvx[0:VW, ds(q0, w)],
                            v_aug[:, ds(t * HPC * VW + (2 * p + hh) * VW, VW)],
                            ste[:, ds(hh * NQ + q0, w)],
                            start=(t == 0), stop=(t == ntile - 1))

                pending = []
                for t in range(ntile):
                    ste = emit_scores_exp(t)
                    # phase-0 special: V chains injected between scores and
                    # AV so the first exps overlap the V projections
                    if local_fill:
                        for _ in range(3):
                            if local_fill:
                                local_fill.pop(0)()
                    pending.append((t, ste))
                    if len(pending) > 1:
                        (pt, pste) = pending.pop(0)
                        emit_av(pt, pste)
                        pop_filler()
                        if phase < 4 and fptr[0] < len(FILL):
                            pop_filler()
                for (pt, pste) in pending:
                    emit_av(pt, pste)
                    pop_filler()

                lcp_a = p_work.tile([1, NQ], F32, tag="lcp_a", bufs=3, name="lcp_a")
                lcp_b = p_work.tile([1, NQ], F32, tag="lcp_b", bufs=3, name="lcp_b")
                recl_a = p_work.tile([1, NQ], F32, tag="recl_a", bufs=3, name="recl_a")
                recl_b = p_work.tile([1, NQ], F32, tag="recl_b", bufs=3, name="recl_b")
                rb_a = p_work.tile([64, NQ], F32, tag="rb_a", bufs=3, name="rb_a")
                rb_b = p_work.tile([64, NQ], F32, tag="rb_b", bufs=3, name="rb_b")
                if (p, j) == (1, 3):
                    # final phase: the whole chain is tail-exposed; emit the
                    # a-chain first so its gpsimd broadcast overlaps the
                    # b-chain's DVE work (other phases keep verified order)
                    nc.vector.tensor_copy(lcp_a, pva[64:65, :])
                    nc.vector.reciprocal_approx_fast(recl_a, lcp_a)
                    nc.gpsimd.partition_broadcast(rb_a, recl_a, channels=64)
                    nc.vector.tensor_copy(lcp_b, pvb[64:65, :])
                    nc.vector.reciprocal_approx_fast(recl_b, lcp_b)
                    nc.gpsimd.partition_broadcast(rb_b, recl_b, channels=64)
                else:
                    nc.vector.tensor_copy(lcp_a, pva[64:65, :])
                    nc.vector.tensor_copy(lcp_b, pvb[64:65, :])
                    nc.vector.reciprocal_approx_fast(recl_a, lcp_a)
                    nc.vector.reciprocal_approx_fast(recl_b, lcp_b)
                    nc.gpsimd.partition_broadcast(rb_a, recl_a, channels=64)
                    nc.gpsimd.partition_broadcast(rb_b, recl_b, channels=64)
                sl = ds(j * NQ, NQ)
                nc.vector.tensor_mul(attnT[p][0:64, sl], pva[0:64, :], rb_a)
                nc.vector.tensor_mul(attnT[p][64:128, sl], pvb[0:64, :], rb_b)

            emit_b0()
            v03 = []
            for sti in range(4):
                v03 += v_pieces(0, sti)
            phase = 0
            for j in range(4):
                for p in range(2):
                    emit_attn(p, j, phase,
                              local_fill=v03 if (p == 0 and j == 0) else None)
                    phase += 1
            # tail: e(3) on the (now free) big pool, psum evict on the (now
            # idle) scalar engine, plus any unconsumed fillers
            drain(99)
            for sti in range(12, 16):
                py2 = ps.tile([128, 1024], F32, tag="big", bufs=2, name="py2")
                for e2 in range(2):
                    for c in range(2):
                        nc.tensor.matmul(
                            py2[:, ds(e2 * NQ, NQ)],
                            attnT[c][:, ds(sti * 128, 128)],
                            wo_sb[:, ds(c * D_MODEL + e2 * NQ, NQ)],
                            start=(c == 0), stop=(c == 1))
                ysb = p_work.tile([128, 1024], BF16, tag="ysb", bufs=3,
                                  name="ysb")
                if sti % 2 == 0:
                    nc.scalar.copy(ysb, py2)
                else:
                    nc.vector.tensor_copy(ysb, py2)
                eng = (nc.sync, nc.gpsimd, nc.scalar, nc.sync)[sti - 12]
                eng.dma_start(out=y[sti * 128:(sti + 1) * 128, :], in_=ysb)
    nc.finalize()
    return nc


def _host_prep(x, Wq, Wk, Wv, Wo):
    x = np.asarray(x, dtype=np.float32)
    Wq, Wk, Wv, Wo = (np.asarray(w, dtype=np.float32) for w in (Wq, Wk, Wv, Wo))
    bf = ml_dtypes.bfloat16

    p64 = np.concatenate([np.arange(0, DK, 2), np.arange(1, DK, 2)])
    freqs = 1.0 / THETA ** (np.arange(0, DK, 2, dtype=np.float64) / DK)
    ang = np.arange(S, dtype=np.float64)[None, :] * freqs[:, None]
    cos32 = np.cos(ang).astype(np.float32)
    sin32 = np.sin(ang).astype(np.float32)
    cosT = np.ascontiguousarray(np.tile(cos32, (4, 1))).astype(bf)
    # sin_alt: pre-swap layout [s, -s, s, -s]; multiply first, then swap
    # 32-blocks, landing [-s, s, -s, s] contributions.
    sinT = np.ascontiguousarray(
        np.concatenate([sin32, -sin32, sin32, -sin32], axis=0)).astype(bf)

    xTs = [np.ascontiguousarray(x[b].T).astype(bf) for b in range(B)]
    perm = np.concatenate([h * DK + p64 for h in range(HPC)])

    in_maps = []
    for core in range(8):
        bg, hg = core // 4, core % 4
        sl = slice(hg * EL, (hg + 1) * EL)
        in_maps.append({
            "xT": xTs[bg],
            "wq": np.ascontiguousarray(Wq[sl][perm].T).astype(bf),
            "wk": np.ascontiguousarray(Wk[sl][perm].T).astype(bf),
            "wv": np.ascontiguousarray(Wv[sl].T).astype(bf),
            "wo": np.ascontiguousarray(Wo[:, sl].T).astype(bf),
            "cosT": cosT,
            "sinT": sinT,
        })
    return in_maps


def kernel(x, Wq, Wk, Wv, Wo, _trace=False):
    if "nc" not in _CACHE:
        _CACHE["nc"] = _build_nc()
    nc = _CACHE["nc"]
    in_maps = _host_prep(x, Wq, Wk, Wv, Wo)
    res = run_bass_kernel_spmd(nc, in_maps, core_ids=list(range(8)), trace=_trace)
    _CACHE["last_result"] = res
    out = np.zeros((B, S, D_MODEL), dtype=np.float32)
    for core in range(8):
        out[core // 4] += np.asarray(res.results[core]["y"], dtype=np.float32)
    return out
